# revision 46
# baseline (speedup 1.0000x reference)
"""DDiT block kernel for 8 Trainium2 NeuronCores — fp8 DoubleRow edition.

Sharding: core = (batch b = core//2, seq half = core%2). Each core computes
q/attention/MLP for its own 1024 tokens and k/v for all 2048 (redundant
compute instead of a collective). Host-side input rotation puts each core's
own tokens in columns 0:1024 so one SPMD program serves all cores.

Key speed mechanisms vs the bf16 baseline:
  - QKV / scores / attn@V / out-proj matmuls run fp8(e4m3) DoubleRow
    (2 K-slots per pass, 0.5 cyc/row). Weights are host-scaled by 64; the
    scale is folded into the rotary tables (q,k), the v PSUM copy, and the
    g_msa adaLN column (out-proj). MLP stays bf16 (fp8 there costs ~1e-2
    rel err). Scores isolate heads by zero-padding the q operand per head;
    the k operand's second K-slot then just reads the next k-block.
  - Softmax: ACT exp -> fp8 E tiles; denominator via a ones-column in V,
    reciprocal_approx_fast on DVE — no Ln/Exp ACT-table thrash. attn@V
    accumulates all 16 k-blocks in PSUM.
  - The exp stream (~200us on ACT at 1 elem/cycle/lane) is the wall; the
    kernel is organized so ACT never idles: attention for query block 0
    is interleaved into the k-projection loop (per head-pair, right after
    that pair's rope), and query block 1's attention sweep carries the
    out-proj/LN2/MLP of block 0 between its head-pair segments.
  - PSUM-evacuation copies run on ACT in the QKV phase (PE-light), LN
    rstd = ACT Sqrt + DVE reciprocal (no table thrash vs exp).
"""

import numpy as np
import sys

sys.path.insert(0, "/opt/trn_rl_repo")

B, S, D, H, DH = 4, 2048, 768, 12, 64
COND, MLP = 128, 3072
EPS = 1e-5
P = 128
SH = S // 2          # tokens per core (1024)
DK = D // P          # 6 feature chunks
MK = MLP // P        # 24 mlp chunks
N_CORES = 8
WS = 64.0            # fp8 weight scale

_prog_cache = {}


def _build_program():
    import concourse.tile as tile
    from concourse import bacc
    import concourse.mybir as mybir
    from contextlib import ExitStack

    f32 = mybir.dt.float32
    bf16 = mybir.dt.bfloat16
    fp8 = mybir.dt.float8e4
    AF = mybir.ActivationFunctionType
    OP = mybir.AluOpType
    PM = mybir.MatmulPerfMode

    nc = bacc.Bacc("TRN2", target_bir_lowering=False, debug=False,
                   enable_asserts=False, num_devices=N_CORES)

    # ---- DRAM I/O (per-core shapes) ----
    xT16_d = nc.dram_tensor("xT16", [P, DK, S], bf16, kind="ExternalInput").ap()
    xo_d = nc.dram_tensor("xTown", [P, DK, SH], f32, kind="ExternalInput").ap()
    c_d = nc.dram_tensor("cT", [COND, 1], bf16, kind="ExternalInput").ap()
    cos_d = nc.dram_tensor("cos4", [P, S], bf16, kind="ExternalInput").ap()
    sin_d = nc.dram_tensor("sin4", [P, S], bf16, kind="ExternalInput").ap()
    wada_d = nc.dram_tensor("WadaT", [COND, 6 * D], bf16, kind="ExternalInput").ap()
    bada_d = nc.dram_tensor("badaT", [P, 36], f32, kind="ExternalInput").ap()
    ln1w_d = nc.dram_tensor("ln1wT", [P, DK], f32, kind="ExternalInput").ap()
    ln2w_d = nc.dram_tensor("ln2wT", [P, DK], f32, kind="ExternalInput").ap()
    wqk_d = nc.dram_tensor("Wqk8", [2 * DK, P, DK, P], fp8, kind="ExternalInput").ap()
    wv_d = nc.dram_tensor("Wv8", [P, DK, D], fp8, kind="ExternalInput").ap()
    wout_d = nc.dram_tensor("Wo8", [DK, P, DK, P], fp8, kind="ExternalInput").ap()
    w1_d = nc.dram_tensor("W18", [MK, P, DK, P], fp8, kind="ExternalInput").ap()
    b1_d = nc.dram_tensor("b1T", [P, MK], f32, kind="ExternalInput").ap()
    w2_d = nc.dram_tensor("W2B", [DK, P, MK, P], bf16, kind="ExternalInput").ap()
    b2_d = nc.dram_tensor("b2T", [P, DK], f32, kind="ExternalInput").ap()
    out_d = nc.dram_tensor("outT", [D, SH], f32, kind="ExternalOutput").ap()

    NQ = H // 2   # 6 head pairs
    KC = S // P   # 16 key blocks

    with tile.TileContext(nc) as tc, ExitStack() as ctx:
        base = ctx.enter_context(tc.tile_pool(name="base", bufs=1))
        stat = ctx.enter_context(tc.tile_pool(name="stat", bufs=1))
        bcast = ctx.enter_context(tc.tile_pool(name="bcast", bufs=2))
        sqp = ctx.enter_context(tc.tile_pool(name="sqp", bufs=2))
        wpool = ctx.enter_context(tc.tile_pool(name="wpool", bufs=3))
        persist = ctx.enter_context(tc.tile_pool(name="persist", bufs=1))
        e2p = ctx.enter_context(tc.tile_pool(name="e2p", bufs=4))
        fin = ctx.enter_context(tc.tile_pool(name="fin", bufs=2))

        # persistent attention operands
        qT2 = [persist.tile([P, 2, SH], fp8, name=f"qT2_{h}") for h in range(H)]
        kZ = [persist.tile([P, S + P], fp8, name=f"kZ{m}") for m in range(NQ)]
        vA2 = [persist.tile([P, 2, H, 72], fp8, name=f"vA2_{t}")
               for t in range(KC // 2)]
        oTs = persist.tile([P, DK, SH], fp8, name="oTs")
        for h in range(H):
            nc.gpsimd.memset(qT2[h][:], 0.0)
        for m in range(NQ):
            nc.gpsimd.memset(kZ[m][:, S:S + P], 0.0)
        for t in range(KC // 2):
            nc.vector.memset(vA2[t][:, :, :, DH:DH + 1], 1.0)

        ones = base.tile([P, 1], bf16, name="ones")
        nc.vector.memset(ones[:], 1.0)
        epsT = base.tile([1, 1], f32, name="epsT")
        nc.vector.memset(epsT[:], EPS)
        ada = base.tile([P, 36], f32, name="ada")
        ln1s = base.tile([P, DK], f32, name="ln1s")
        ln2s = base.tile([P, DK], f32, name="ln2s")
        b1s = base.tile([P, MK], f32, name="b1s")
        nc.sync.dma_start(b1s[:], b1_d[:, :])
        b2s = base.tile([P, DK], f32, name="b2s")
        nc.sync.dma_start(b2s[:], b2_d[:, :])

        def ln_stats(psp, src_cols, n, sq_on_act=False):
            """sum & sumsq matmuls over DK chunks -> broadcast (rstd, mean).
            Stats live in row 0 of [P,512]-shaped tiles sharing tag "mm"."""
            ps_s = psp.tile([P, 512], f32, tag="mm", name="ps_s")
            ps_q = psp.tile([P, 512], f32, tag="mm", name="ps_q")
            for k in range(DK):
                nc.tensor.matmul(ps_s[0:1, 0:n], ones[:], src_cols[k],
                                 start=(k == 0), stop=(k == DK - 1))
                sq = sqp.tile([P, 512], bf16, tag="sq", name="sq")
                if sq_on_act:
                    nc.scalar.square(sq[:, 0:n], src_cols[k])
                else:
                    nc.vector.tensor_mul(sq[:, 0:n], src_cols[k], src_cols[k])
                nc.tensor.matmul(ps_q[0:1, 0:n], ones[:], sq[:, 0:n],
                                 start=(k == 0), stop=(k == DK - 1))
            mean = stat.tile([1, 512], f32, tag="mean", name="mean")
            nc.vector.tensor_scalar_mul(mean[0:1, 0:n], ps_s[0:1, 0:n], 1.0 / D)
            var = stat.tile([1, 512], f32, tag="var", name="var")
            nc.vector.tensor_scalar_mul(var[0:1, 0:n], ps_q[0:1, 0:n], 1.0 / D)
            aux = stat.tile([1, 512], f32, tag="aux", name="aux")
            nc.vector.tensor_mul(aux[0:1, 0:n], mean[0:1, 0:n], mean[0:1, 0:n])
            nc.vector.tensor_sub(var[0:1, 0:n], var[0:1, 0:n], aux[0:1, 0:n])
            sd = stat.tile([1, 512], f32, tag="sd", name="sd")
            nc.scalar.activation(sd[0:1, 0:n], var[0:1, 0:n], AF.Sqrt,
                                 bias=epsT[:])
            rstdf = stat.tile([1, 512], f32, tag="rstdf", name="rstdf")
            nc.vector.reciprocal_approx_fast(rstdf[0:1, 0:n], sd[0:1, 0:n])
            rstdb = stat.tile([1, 512], bf16, tag="rstdb", name="rstdb")
            nc.vector.tensor_copy(rstdb[0:1, 0:n], rstdf[0:1, 0:n])
            meanb = stat.tile([1, 512], bf16, tag="meanb", name="meanb")
            nc.vector.tensor_copy(meanb[0:1, 0:n], mean[0:1, 0:n])
            A128 = bcast.tile([P, 512], bf16, tag="A128", name="A128")
            B128 = bcast.tile([P, 512], bf16, tag="B128", name="B128")
            nc.gpsimd.partition_broadcast(A128[:, 0:n], rstdb[0:1, 0:n])
            nc.gpsimd.partition_broadcast(B128[:, 0:n], meanb[0:1, 0:n])
            return A128, B128

        def ln_modulate(src_cols, A128, B128, scale_cols, shift_col0, dst,
                        dst_sl, n):
            for k in range(DK):
                t2 = sqp.tile([P, 512], bf16, tag="t2", name="t2")
                nc.vector.tensor_sub(t2[:, 0:n], src_cols[k], B128[:, 0:n])
                nc.vector.tensor_mul(t2[:, 0:n], t2[:, 0:n], A128[:, 0:n])
                nc.vector.tensor_scalar(
                    dst[:, k, dst_sl], t2[:, 0:n],
                    scale_cols[:, k:k + 1], ada[:, shift_col0 + k:shift_col0 + k + 1],
                    OP.mult, OP.add)

        # ---- attention sweep machinery (segmented, cross-segment lookahead) --
        class Sweep:
            def __init__(self, qb, psS, psO):
                self.qb = qb
                self.psS = psS
                self.psO = psO
                self.sgq = {}

            def emit_S(self, p, kc):
                sg = self.psS.tile([P, 2, 512], f32, tag="sg", name="sg")
                kap = kZ[p][:, kc * P:(kc + 2) * P].rearrange(
                    "a (two c) -> a two c", two=2)
                qsl = slice(self.qb * 512, (self.qb + 1) * 512)
                for hh in range(2):
                    nc.tensor.matmul(sg[:, hh, :], kap,
                                     qT2[2 * p + hh][:, :, qsl],
                                     start=True, stop=True,
                                     perf_mode=PM.DoubleRow)
                self.sgq[(p, kc)] = sg

            def ensure(self, p, kc):
                if (p, kc) not in self.sgq:
                    self.emit_S(p, kc)

            def segment(self, p, look_next=False, finalize_cb=None, work=None,
                        pop_at=(1, 3, 5, 7), pop_n=2):
                qb = self.qb
                qsl = slice(qb * 512, (qb + 1) * 512)
                self.ensure(p, 0)
                self.ensure(p, 1)
                oag = [self.psO.tile([DH + 1, 512], f32, tag=f"oag{hh}",
                                     name=f"oag{hh}") for hh in range(2)]
                E2 = None
                for kc in range(KC):
                    sg = self.sgq.pop((p, kc))
                    if kc % 2 == 0:
                        E2 = e2p.tile([P, 2, 2, 512], fp8, tag="E2", name="E2")
                    nc.scalar.activation(E2[:, kc % 2, :, :], sg[:], AF.Exp,
                                         scale=0.125)
                    nxt = kc + 2
                    if nxt < KC:
                        self.ensure(p, nxt)
                    elif look_next and p + 1 < NQ:
                        self.ensure(p + 1, nxt - KC)
                    if kc % 2 == 1:
                        kcp = kc // 2
                        for hh in range(2):
                            nc.tensor.matmul(
                                oag[hh][:], vA2[kcp][:, :, 2 * p + hh, 0:DH + 1],
                                E2[:, :, hh, :],
                                start=(kcp == 0), stop=(kcp == KC // 2 - 1),
                                perf_mode=PM.DoubleRow)
                        if work and kcp in pop_at:
                            budget = pop_n
                            while work and budget > 0:
                                cost, fn = work[0]
                                if budget < pop_n and cost > budget:
                                    break
                                work.pop(0)
                                fn()
                                budget -= cost
                for hh in range(2):
                    # full-tile copy releases the PSUM bank at once so the
                    # next segment's attn@V isn't blocked by this chain
                    d64 = fin.tile([DH + 1, 512], f32, tag="d64", name="d64")
                    nc.vector.tensor_copy(d64[:], oag[hh][:])
                    d0 = fin.tile([1, 512], f32, tag="d0", name="d0")
                    nc.sync.dma_start(d0[:], d64[DH:DH + 1, :])
                    dr = fin.tile([1, 512], f32, tag="dr", name="dr")
                    nc.vector.reciprocal_approx_fast(dr[:], d0[:])
                    rb = fin.tile([DH, 512], f32, tag="rb", name="rb")
                    nc.gpsimd.partition_broadcast(rb[:], dr[:])
                    ot = fin.tile([DH, 512], fp8, tag="ot", name="ot")
                    nc.vector.tensor_mul(ot[:], d64[0:DH, :], rb[:])
                    nc.sync.dma_start(oTs[hh * DH:(hh + 1) * DH, p, qsl], ot[:])
                if finalize_cb is not None:
                    finalize_cb(p)

        # ============ attention-PSUM scope spans QKV + both sweeps ===========
        with tc.tile_pool(name="psS", bufs=2, space="PSUM") as psS, \
             tc.tile_pool(name="psO", bufs=1, space="PSUM") as psO:

            sweep0 = Sweep(0, psS, psO)

            with tc.tile_pool(name="qkvp", bufs=1) as qp, \
                 tc.tile_pool(name="xbp", bufs=4) as xbp, \
                 tc.tile_pool(name="swp", bufs=2) as swp:

                # --- prefetch x blocks before the big weight DMAs ---
                xbs = []
                for i in range(S // 512):
                    xb = xbp.tile([P, DK, 512], bf16, tag="xb", name="xb")
                    nc.sync.dma_start(xb[:], xT16_d[:, :, i * 512:(i + 1) * 512])
                    xbs.append(xb)

                cT = qp.tile([COND, 1], bf16, name="cT")
                nc.sync.dma_start(cT[:], c_d[:, :])
                wada = qp.tile([COND, 6 * D], bf16, name="wada")
                nc.sync.dma_start(wada[:], wada_d[:, :])
                wqk = qp.tile([P, 2 * DK, DK, P], fp8, name="wqk")
                nc.sync.dma_start(wqk[:], wqk_d.rearrange("m p k c -> p m k c"))
                wv = qp.tile([P, DK, D], fp8, name="wv")
                nc.sync.dma_start(wv[:], wv_d[:, :, :])
                cosT = qp.tile([P, S], bf16, name="cosT")
                sinT = qp.tile([P, S], bf16, name="sinT")
                nc.sync.dma_start(cosT[:], cos_d[:, :])
                nc.sync.dma_start(sinT[:], sin_d[:, :])

                hb = qp.tile([P, DK, S], fp8, name="hb")

                with tc.tile_pool(name="psLN", bufs=2, space="PSUM") as psLN:
                    # --- adaLN (shares the psLN "mm" bank ring) ---
                    for jb in range(6):
                        ps = psLN.tile([P, 512], f32, tag="mm", name="ps_ada")
                        for j in range(6):
                            # one group, disjoint columns -> each written once
                            nc.tensor.matmul(ps[:, j:j + 1],
                                             wada[:, (6 * jb + j) * P:
                                                  (6 * jb + j + 1) * P],
                                             cT[:], start=(j == 0),
                                             stop=(j == 5))
                        nc.vector.tensor_copy(ada[:, 6 * jb:6 * jb + 6],
                                              ps[:, 0:6])
                    badaT = qp.tile([P, 36], f32, name="badaT")
                    nc.sync.dma_start(badaT[:], bada_d[:, :])
                    # ln1s only depends on ada cols 0:12 -> LN1 starts early
                    nc.vector.tensor_add(ada[:, 0:12], ada[:, 0:12],
                                         badaT[:, 0:12])
                    nc.vector.tensor_scalar_add(ada[:, 6:12], ada[:, 6:12], 1.0)
                    lw = qp.tile([P, DK], f32, name="lnw1")
                    nc.sync.dma_start(lw[:], ln1w_d[:, :])
                    nc.vector.tensor_mul(ln1s[:], lw[:], ada[:, 6:12])
                    nc.vector.tensor_add(ada[:, 12:36], ada[:, 12:36],
                                         badaT[:, 12:36])
                    nc.vector.tensor_scalar_add(ada[:, 24:30], ada[:, 24:30], 1.0)
                    nc.vector.tensor_scalar_mul(ada[:, 12:18], ada[:, 12:18],
                                                1.0 / WS)
                    lw2 = qp.tile([P, DK], f32, name="lnw2")
                    nc.sync.dma_start(lw2[:], ln2w_d[:, :])
                    nc.vector.tensor_mul(ln2s[:], lw2[:], ada[:, 24:30])

                    # --- LN1 over all 2048 tokens -> hb fp8 ---
                    for i in range(S // 512):
                        cols = [xbs[i][:, k, :] for k in range(DK)]
                        A128, B128 = ln_stats(psLN, cols, 512, sq_on_act=True)
                        ln_modulate(cols, A128, B128, ln1s, 0, hb,
                                    slice(i * 512, (i + 1) * 512), 512)

                with tc.tile_pool(name="psQ", bufs=2, space="PSUM") as psQ:
                    # --- q + rope per head-pair (needs only hb blocks 0,1)
                    qT = [qp.tile([P, SH], bf16, name=f"qT{m}")
                          for m in range(NQ)]
                    for m in range(NQ):
                        for i in range(2):
                            ps = psQ.tile([P, 512], f32, tag="mm", name="ps_q")
                            for j in range(DK // 2):
                                nc.tensor.matmul(
                                    ps[:], wqk[:, m, 2 * j:2 * j + 2, :],
                                    hb[:, 2 * j:2 * j + 2, i * 512:(i + 1) * 512],
                                    start=(j == 0), stop=(j == DK // 2 - 1),
                                    perf_mode=PM.DoubleRow)
                            nc.scalar.copy(qT[m][:, i * 512:(i + 1) * 512],
                                           ps[:])
                        sw = swp.tile([P, SH], bf16, tag="swq", name="swq")
                        t = qT[m]
                        nc.sync.dma_start(sw[0:32, :], t[32:64, :])
                        nc.sync.dma_start(sw[32:64, :], t[0:32, :])
                        nc.sync.dma_start(sw[64:96, :], t[96:128, :])
                        nc.sync.dma_start(sw[96:128, :], t[64:96, :])
                        nc.vector.tensor_mul(t[:], t[:], cosT[:, 0:SH])
                        nc.vector.tensor_mul(sw[:], sw[:], sinT[:, 0:SH])
                        nc.vector.tensor_add(qT2[2 * m][0:DH, 0, :], t[0:DH, :],
                                             sw[0:DH, :])
                        nc.vector.tensor_add(qT2[2 * m + 1][DH:P, 0, :],
                                             t[DH:P, :], sw[DH:P, :])

                    # --- v (all 2048 tokens), hb stationary; t>=4 woven
                    # into attention segment 0, one kc-pair ahead of use ---
                    def v_item(t):
                        ps1 = psQ.tile([P, 512], f32, tag="mm", name="ps_v1")
                        ps2 = psQ.tile([P, 512], f32, tag="mm", name="ps_v2")
                        for j in range(DK // 2):
                            lhs = hb[:, 2 * j:2 * j + 2, t * P:(t + 1) * P]
                            nc.tensor.matmul(ps1[:], lhs,
                                             wv[:, 2 * j:2 * j + 2, 0:512],
                                             start=(j == 0),
                                             stop=(j == DK // 2 - 1),
                                             perf_mode=PM.DoubleRow)
                            nc.tensor.matmul(ps2[:, 0:256], lhs,
                                             wv[:, 2 * j:2 * j + 2, 512:768],
                                             start=(j == 0),
                                             stop=(j == DK // 2 - 1),
                                             perf_mode=PM.DoubleRow)
                        nc.scalar.mul(
                            vA2[t // 2][:, t % 2, 0:8, 0:DH],
                            ps1[:].rearrange("p (h d) -> p h d", d=DH), 1.0 / WS)
                        nc.scalar.mul(
                            vA2[t // 2][:, t % 2, 8:H, 0:DH],
                            ps2[:, 0:256].rearrange("p (h d) -> p h d", d=DH),
                            1.0 / WS)

                    for t in range(4):
                        v_item(t)
                    vwork = [(1.0, lambda t=t: v_item(t))
                             for t in range(4, KC)]

                    # --- k per head-pair, roped per 512-col block (kZ[m]
                    # fills as each LN1 block lands), then pair m's qb0
                    # attention; pair 0's segment also emits v t=4..15
                    for m in range(NQ):
                        kt = swp.tile([P, S], bf16, tag="kt", name=f"kt{m}")
                        for i in range(S // 512):
                            csl = slice(i * 512, (i + 1) * 512)
                            ps = psQ.tile([P, 512], f32, tag="mm", name="ps_k")
                            for j in range(DK // 2):
                                nc.tensor.matmul(
                                    ps[:], wqk[:, DK + m, 2 * j:2 * j + 2, :],
                                    hb[:, 2 * j:2 * j + 2, csl],
                                    start=(j == 0), stop=(j == DK // 2 - 1),
                                    perf_mode=PM.DoubleRow)
                            nc.vector.tensor_copy(kt[:, csl], ps[:])
                            sw = swp.tile([P, 512], bf16, tag="swk", name="swk")
                            nc.sync.dma_start(sw[0:32, :], kt[32:64, csl])
                            nc.sync.dma_start(sw[32:64, :], kt[0:32, csl])
                            nc.sync.dma_start(sw[64:96, :], kt[96:128, csl])
                            nc.sync.dma_start(sw[96:128, :], kt[64:96, csl])
                            nc.vector.tensor_mul(kt[:, csl], kt[:, csl],
                                                 cosT[:, csl])
                            nc.vector.tensor_mul(sw[:], sw[:], sinT[:, csl])
                            nc.vector.tensor_add(kZ[m][:, csl], kt[:, csl],
                                                 sw[:])
                        sweep0.segment(m, look_next=False,
                                       work=vwork if m == 0 else None,
                                       pop_at=(0, 1, 2, 3, 4, 5), pop_n=2.0)

            # ============ qb1 attention with qb0's out/LN2/MLP woven in ======
            with tc.tile_pool(name="psM", bufs=2, space="PSUM") as psM, \
                 tc.tile_pool(name="mlp_ph", bufs=1) as mp, \
                 tc.tile_pool(name="m16p", bufs=1) as m16p, \
                 tc.tile_pool(name="w24p", bufs=2) as w24p, \
                 tc.tile_pool(name="mlp_tmp", bufs=2) as mt:

                x1 = mp.tile([P, DK, SH], f32, name="x1")
                h2 = mp.tile([P, DK, SH], fp8, name="h2")
                x16b = mp.tile([P, DK, 512], bf16, name="x16b")

                def outproj_item(i, m):
                    isl = slice(i * 512, (i + 1) * 512)
                    w6 = wpool.tile([P, DK, P], fp8, tag="w6o", name="w6o")
                    nc.sync.dma_start(w6[:], wout_d[m])
                    ps2 = psM.tile([P, 512], f32, tag="mm", name="ps_o")
                    for j in range(DK // 2):
                        nc.tensor.matmul(ps2[:], w6[:, 2 * j:2 * j + 2, :],
                                         oTs[:, 2 * j:2 * j + 2, isl],
                                         start=(j == 0),
                                         stop=(j == DK // 2 - 1),
                                         perf_mode=PM.DoubleRow)
                    xo = mt.tile([P, 512], f32, tag="xo", name="xo")
                    nc.sync.dma_start(xo[:], xo_d[:, m, isl])
                    nc.vector.scalar_tensor_tensor(
                        x1[:, m, isl], ps2[:], ada[:, 12 + m:13 + m], xo[:],
                        OP.mult, OP.add)

                def emit_outproj(i):
                    for m in range(DK):
                        outproj_item(i, m)

                def emit_ln2(i):
                    isl = slice(i * 512, (i + 1) * 512)
                    cols = []
                    for k in range(DK):
                        nc.vector.tensor_copy(x16b[:, k, :], x1[:, k, isl])
                        cols.append(x16b[:, k, :])
                    A128, B128 = ln_stats(psM, cols, 512)
                    ln_modulate(cols, A128, B128, ln2s, 18, h2, isl, 512)

                state = {}

                pre8 = mp.tile([P, 8, 512], bf16, name="pre8")

                def mlp1_item(i, m):
                    """mlp1 matmul + biased PSUM copy; gelu deferred into one
                    batched ACT call per 8 blocks (fewer exp<->gelu table
                    swaps on the saturated ACT engine)."""
                    isl = slice(i * 512, (i + 1) * 512)
                    if m == 0:
                        state["m16"] = m16p.tile([P, MK, 512], bf16,
                                                 tag="m16", name="m16")
                    m16 = state["m16"]
                    w6 = wpool.tile([P, DK, P], fp8, tag="w6m", name="w6m")
                    nc.sync.dma_start(w6[:], w1_d[m])
                    ps = psM.tile([P, 512], f32, tag="mm", name="ps_m")
                    for j in range(DK // 2):
                        nc.tensor.matmul(ps[:], w6[:, 2 * j:2 * j + 2, :],
                                         h2[:, 2 * j:2 * j + 2, isl],
                                         start=(j == 0), stop=(j == DK // 2 - 1),
                                         perf_mode=PM.DoubleRow)
                    nc.vector.tensor_scalar(pre8[:, m % 8, :], ps[:],
                                            1.0 / WS, b1s[:, m:m + 1],
                                            OP.mult, OP.add)
                    if m % 8 == 7:
                        nc.scalar.activation(m16[:, m - 7:m + 1, :], pre8[:],
                                             AF.Gelu_apprx_tanh)

                def mlp2_item(i, m):
                    isl = slice(i * 512, (i + 1) * 512)
                    m16 = state["m16"]
                    w24 = w24p.tile([P, MK, P], bf16, tag="w24", name="w24")
                    nc.sync.dma_start(w24[:], w2_d[m])
                    ps = psM.tile([P, 512], f32, tag="mm", name="ps_y")
                    for k in range(MK):
                        nc.tensor.matmul(ps[:], w24[:, k, :], m16[:, k, :],
                                         start=(k == 0), stop=(k == MK - 1))
                    yt = mt.tile([P, 512], f32, tag="yt", name="yt")
                    nc.vector.tensor_scalar(yt[:], ps[:], b2s[:, m:m + 1],
                                            ada[:, 30 + m:31 + m], OP.add,
                                            OP.mult)
                    nc.vector.tensor_add(yt[:], yt[:], x1[:, m, isl])
                    nc.sync.dma_start(out_d[m * P:(m + 1) * P, isl], yt[:])

                # qb0's out/LN2/MLP as a fine-grained work queue, one item per
                # kc-pair slot of the qb1 sweep (keeps the PE FIFO from
                # blocking the exp stream with big lumps)
                work = []
                for m in range(DK):
                    work.append((0.6, lambda m=m: outproj_item(0, m)))
                work.append((2.5, lambda: emit_ln2(0)))
                for m in range(MK):
                    work.append((0.6, lambda m=m: mlp1_item(0, m)))
                for m in range(DK):
                    work.append((5.1, lambda m=m: mlp2_item(0, m)))

                sweep1 = Sweep(1, psS, psO)
                for p in range(NQ):
                    sweep1.segment(p, look_next=True, work=work,
                                   pop_at=(1, 3, 5, 7), pop_n=2.6)

                while work:
                    work.pop(0)[1]()
                emit_outproj(1)
                emit_ln2(1)
                for m in range(MK):
                    mlp1_item(1, m)
                for m in range(DK):
                    mlp2_item(1, m)

    nc.compile()
    return nc


def _host_prep(inputs):
    """Build per-core in_maps (host-side sharding + layout transforms)."""
    import ml_dtypes
    bf16 = ml_dtypes.bfloat16
    f8 = ml_dtypes.float8_e4m3fn

    x = np.ascontiguousarray(inputs["x"], dtype=np.float32)
    cos = np.asarray(inputs["cos"], dtype=np.float32)
    sin = np.asarray(inputs["sin"], dtype=np.float32)
    c = np.asarray(inputs["c"], dtype=np.float32)

    cos_s = cos[0, :, 0, 0, :DH // 2]      # (S, 32)
    sin_s = sin[0, :, 0, 0, :DH // 2]
    # C4[p, t] = cos_s[t, p%32]; S4 sign-folded; both carry the 1/WS fold
    pidx = np.arange(P)
    C4 = cos_s.T[pidx % 32, :] * (1.0 / WS)
    sgn = np.where((pidx % 64) < 32, -1.0, 1.0).astype(np.float32)
    S4 = sin_s.T[pidx % 32, :] * sgn[:, None] * (1.0 / WS)

    WadaT = np.ascontiguousarray(inputs["W_ada"].T).astype(bf16)        # (128, 4608)
    badaT = np.ascontiguousarray(
        np.asarray(inputs["b_ada"], np.float32).reshape(36, P).T)       # (128, 36)

    def blocks(wT, nblk, dt, scale=1.0):
        K, N = wT.shape
        w = wT * scale
        if dt is f8:
            w = np.clip(w, -240.0, 240.0)
        return np.ascontiguousarray(
            w.reshape(K // P, P, nblk, P).transpose(2, 1, 0, 3)).astype(dt)

    WqkvT = inputs["W_qkv"].T.astype(np.float32)                        # (768, 2304)
    Wqk8 = blocks(WqkvT[:, :2 * D], 2 * DK, f8, WS)                     # (12,128,6,128)
    Wv8 = np.ascontiguousarray(
        np.clip(WqkvT[:, 2 * D:] * WS, -240, 240)
        .reshape(DK, P, D).transpose(1, 0, 2)).astype(f8)               # (128,6,768)
    Wo8 = blocks(inputs["W_out"].T.astype(np.float32), DK, f8, WS)
    W18 = blocks(inputs["W_mlp1"].T.astype(np.float32), MK, f8, WS)
    W2B = blocks(inputs["W_mlp2"].T.astype(np.float32), DK, bf16)
    b1T = np.ascontiguousarray(
        np.asarray(inputs["b_mlp1"], np.float32).reshape(MK, P).T)
    b2T = np.ascontiguousarray(
        np.asarray(inputs["b_mlp2"], np.float32).reshape(DK, P).T)
    ln1wT = np.ascontiguousarray(
        np.asarray(inputs["ln1_w"], np.float32).reshape(DK, P).T)
    ln2wT = np.ascontiguousarray(
        np.asarray(inputs["ln2_w"], np.float32).reshape(DK, P).T)

    in_maps = []
    for core in range(N_CORES):
        b, half = core // 2, core % 2
        own = slice(half * SH, half * SH + SH)
        oth = slice((1 - half) * SH, (1 - half) * SH + SH)
        xb = x[b]                                            # (S, D)
        xT = np.concatenate([xb[own].T, xb[oth].T], axis=1)  # (768, 2048)
        cos4 = np.concatenate([C4[:, own], C4[:, oth]], axis=1).astype(bf16)
        sin4 = np.concatenate([S4[:, own], S4[:, oth]], axis=1).astype(bf16)
        xT16 = np.ascontiguousarray(
            xT.reshape(DK, P, S).transpose(1, 0, 2)).astype(bf16)
        xTown = np.ascontiguousarray(
            xT[:, 0:SH].reshape(DK, P, SH).transpose(1, 0, 2))
        in_maps.append({
            "xT16": xT16,
            "xTown": xTown,
            "cT": np.ascontiguousarray(c[b].reshape(COND, 1)).astype(bf16),
            "cos4": np.ascontiguousarray(cos4),
            "sin4": np.ascontiguousarray(sin4),
            "WadaT": WadaT, "badaT": badaT,
            "ln1wT": ln1wT, "ln2wT": ln2wT,
            "Wqk8": Wqk8, "Wv8": Wv8, "Wo8": Wo8,
            "W18": W18, "b1T": b1T, "W2B": W2B, "b2T": b2T,
        })
    return in_maps


def _get_program():
    if "nc" not in _prog_cache:
        _prog_cache["nc"] = _build_program()
    return _prog_cache["nc"]


def kernel(**inputs):
    from concourse.bass_utils import run_bass_kernel_spmd
    nc = _get_program()
    in_maps = _host_prep(inputs)
    res = run_bass_kernel_spmd(nc, in_maps, core_ids=list(range(N_CORES)))
    out = np.empty((B, S, D), dtype=np.float32)
    for core in range(N_CORES):
        b, half = core // 2, core % 2
        out[b, half * SH:(half + 1) * SH, :] = res.results[core]["outT"].T
    return out
